# revision 9
# baseline (speedup 1.0000x reference)
"""CompressedFP8Linear on 8 trn2 NeuronCores.

out[B,S,O] = x @ (weight * weight_scale).T + bias
  x:[4,32,8192] f32, weight:[8192,8192] f32 (fp8-e4m3 representable),
  weight_scale:[8192,1] f32, bias:[8192] f16.

Strategy (column-parallel, per sharding hint):
  - Shard weight rows (out_features) across 8 cores; replicate x.
  - Host-side marshalling: transpose each weight shard to [K, O_shard]
    (k-major) and x to [K, M] so the PE sees the contraction dim on
    partitions; broadcast scale/bias along the 128 token rows.
  - Per core: out = (xT.T @ WT) * scale + bias, accumulated over 64
    K-tiles of 128 in PSUM.  Matmuls run in float32r (fp32 data, single
    "High"-pass): full PE speed at moving-dim 512, ~1e-4 rel precision,
    and the fp8-representable weights are exact.
  - Per-output-channel dequant scale is applied to the [128, O] output
    (64x fewer multiplies than dequantizing the weight), bias added on
    the vector engine.  No collectives; host concatenates shards.

Memory roofline per core: 32 MiB weight + 4 MiB x + 1 MiB scale/bias
+ 0.5 MiB out at ~360 GB/s  =>  ~105 us.
"""

import numpy as np

import concourse.bass as bass
import concourse.mybir as mybir
import concourse.tile as tile
from concourse.bass_utils import run_bass_kernel_spmd

B, S, IN, OUT = 4, 32, 8192, 8192
M = B * S                      # 128 tokens
NCORES = 8
OSH = OUT // NCORES            # 1024 out-features per core
KT = IN // 128                 # 64 k-tiles
F32 = mybir.dt.float32
F32R = mybir.dt.float32r


def split_waits(nc, max_waits=1):
    """This walrus build encodes at most one sem-wait per instruction;
    move any excess onto NoOps injected just before (same engine queue,
    so ordering semantics are identical)."""
    n = 0
    for f in nc.m.functions:
        for bb in f.blocks:
            out = []
            for inst in bb.instructions:
                si = inst.sync_info
                waits = list(si.on_wait) if si and si.on_wait else []
                if len(waits) > max_waits:
                    extra, keep = waits[:-max_waits], waits[-max_waits:]
                    for i, w in enumerate(extra):
                        out.append(mybir.InstNoOp(
                            name=f"{inst.name}-ws{i}", engine=inst.engine,
                            ins=[], outs=[],
                            sync_info=mybir.SyncInfo(on_wait=[w], on_update=[])))
                        n += 1
                    si.on_wait = keep
                out.append(inst)
            bb.instructions = out
    return n


def build(reps=1, slab_kt=4, w_engines=("sync", "scalar"), x_engine="gpsimd"):
    """One column-parallel shard: out[128, OSH] = xT.T @ WT * scale + bias.

    reps > 1 unrolls the whole body (including all DMA) back-to-back for
    wall-clock timing; the computation is identical each rep.
    """
    nc = bass.Bass()
    # xt is host-packed [p, kt, m]: each partition's 32 KiB is contiguous
    xt_d = nc.dram_tensor("xt", [128, KT, M], F32R, kind="ExternalInput")
    wt_d = nc.dram_tensor("wt", [IN, OSH], F32R, kind="ExternalInput")
    sc_d = nc.dram_tensor("scale_b", [M, OSH], F32, kind="ExternalInput")
    bi_d = nc.dram_tensor("bias_b", [M, OSH], F32, kind="ExternalInput")
    out_d = nc.dram_tensor("out", [M, OSH], F32, kind="ExternalOutput")

    xt3 = xt_d[:]                                               # [128, KT, 128]
    wt3 = wt_d[:].rearrange("(t s p) o -> p t s o", s=slab_kt, p=128)  # [128, KT//slab, slab, OSH]

    with tile.TileContext(nc) as tc:
        with (
            tc.tile_pool(name="xp", bufs=2) as xp,
            tc.tile_pool(name="wp", bufs=4) as wp,
            tc.tile_pool(name="cp", bufs=1) as cp,
            tc.tile_pool(name="op", bufs=2) as op,
            tc.tile_pool(name="ps", bufs=2, space="PSUM") as ps,
        ):
            x_eng = getattr(nc, x_engine)
            w_engs = [getattr(nc, e) for e in w_engines]

            sc = cp.tile([M, OSH], F32)
            bi = cp.tile([M, OSH], F32)
            x_eng.dma_start(sc[:], sc_d[:])
            x_eng.dma_start(bi[:], bi_d[:])

            for _ in range(reps):
                # x: 4 MiB in 8 chunks so the first matmul waits only ~0.5 MiB
                xsb = xp.tile([128, KT, M], F32R)
                per = KT // 8
                for i in range(8):
                    x_eng.dma_start(
                        xsb[:, i * per:(i + 1) * per, :],
                        xt3[:, i * per:(i + 1) * per, :])

                acc0 = ps.tile([M, 512], F32)
                acc1 = ps.tile([M, 512], F32)
                accs = (acc0, acc1)
                for t in range(KT // slab_kt):
                    wsb = wp.tile([128, slab_kt, OSH], F32R)  # slab: k-tiles t*slab..
                    # spread weight DMAs over rings so they pipeline
                    w_engs[t % len(w_engs)].dma_start(wsb[:], wt3[:, t])
                    for s in range(slab_kt):
                        k = slab_kt * t + s
                        for og in range(2):
                            nc.tensor.matmul(
                                accs[og][:, :],
                                xsb[:, k, :],
                                wsb[:, s, og * 512:(og + 1) * 512],
                                start=(k == 0), stop=(k == KT - 1))

                outsb = op.tile([M, OSH], F32)
                for og in range(2):
                    osl = outsb[:, og * 512:(og + 1) * 512]
                    nc.vector.tensor_mul(osl, accs[og][:, :], sc[:, og * 512:(og + 1) * 512])
                    nc.vector.tensor_add(osl, osl, bi[:, og * 512:(og + 1) * 512])
                x_eng.dma_start(out_d[:], outsb[:])

    split_waits(nc)
    return nc


def shard_inputs(x, weight, weight_scale, bias):
    """Host-side marshalling into per-core input maps (layout only)."""
    x = np.asarray(x, dtype=np.float32)
    weight = np.asarray(weight, dtype=np.float32)
    scale = np.asarray(weight_scale, dtype=np.float32).reshape(OUT)
    bias32 = np.asarray(bias).astype(np.float32)

    # pack x as [p, kt, m] (k = kt*128 + p) so each SBUF partition's x data
    # is one contiguous DRAM run
    xt = np.ascontiguousarray(np.transpose(x.reshape(M, KT, 128), (2, 1, 0)))
    in_maps = []
    for c in range(NCORES):
        sl = slice(c * OSH, (c + 1) * OSH)
        wt = np.ascontiguousarray(weight[sl, :].T)              # [IN, OSH]
        sc = np.ascontiguousarray(np.broadcast_to(scale[sl][None, :], (M, OSH)))
        bi = np.ascontiguousarray(np.broadcast_to(bias32[sl][None, :], (M, OSH)))
        in_maps.append({"xt": xt, "wt": wt, "scale_b": sc, "bias_b": bi})
    return in_maps


def kernel(x, weight, weight_scale, bias):
    nc = build(reps=1)
    in_maps = shard_inputs(x, weight, weight_scale, bias)
    res = run_bass_kernel_spmd(nc, in_maps, core_ids=list(range(NCORES)))
    out = np.concatenate([res.results[c]["out"] for c in range(NCORES)], axis=1)
    return out.reshape(B, S, OUT)
